# revision 25
# baseline (speedup 1.0000x reference)
"""BatchAllTripletLoss on 8 Trainium2 NeuronCores (sorted-layout version).

Contract: kernel(**inputs) takes the FULL inputs (embs [512,128] f32,
idtys [512] int64) and returns the FULL output (scalar f32 loss).

Math: d = pairwise euclidean distances [512,512];
  loss = sum_{a,p,n} relu(d[a,p]-d[a,n]+margin)*mask / (num_pos + eps)
The mask factorizes as pos[a,p]*neg[a,n] (pos: same id, p!=a; neg:
different id). All index work happens on the host: samples are sorted
by id so each group is contiguous, and per core the sample order is
rotated so its 128 anchors sit at positions AOFF..AOFF+127 and every
anchor's group lies inside positions [0, 256).

Per-core device program (core c: anchor block b=c//2, parity par=c%2,
handling positive ranks {par, par+2, ...} = KP columns):
 1. d2 rows for the 128 anchors via bf16 PE matmuls (-2*dot, ones@e2
    for sq[n], rank-1 sq[n] broadcast), pipelined over two n-halves;
    sq[a] via DVE square-accumulate; sq[a]-add + relu clamp fused in
    one DVE op; ACT sqrt -> d.
 2. ndneg = -(d + BIG*same) in bf16 (same-mask is a host input; BIG
    pushes same-id negatives out of every relu/count).
 3. x2[a,j] = d2[a, p_j(a)] selected in SQUARED space with NO
    transposes: dotT chunks [n,a] = -2 emb^T emb_A are plain matmuls,
    d2mT = (dotT + sq[n]) * maskT (host-transposed mask), one bf16
    matmul against a host rank-one-hot E, then sq[a]*exist fixup,
    clamp, tiny sqrt, margin/valid. No DRAM round trip, no indirect
    DMA, no device argsort.
 4. Main loop over KP rank columns, engines in parallel: ACT
    Relu(ndneg + x_j) -> t_j; PE ones-matmul accumulates the relu sum
    into PSUM [1,512]; DVE (ndneg > -x_j) fused-accum counts cols
    [0,336); GpSimd counts cols [336,512).
 5. Free-axis reduces -> out [128,3] = (cntA col, cntB col, relu sum
    in row 0); host sums columns across cores and divides.
"""

import numpy as np

B = 512
D = 128
NCORES = 8
AH = 128          # anchors per core
AOFF = 16         # rotated position of the first anchor
NSPL = 336        # count split: DVE does [0,NSPL), GpSimd the rest
MARGIN = 0.2
BIG = 1.0e6

_CACHE = {}


def _build_bass(KP):
    import concourse.bass as bass
    import concourse.tile as tile
    from concourse import mybir

    f32 = mybir.dt.float32
    bf16 = mybir.dt.bfloat16
    AF = mybir.ActivationFunctionType
    OP = mybir.AluOpType
    X = mybir.AxisListType.X

    nc = bass.Bass()

    emT = nc.dram_tensor("emT", [D, B], bf16, kind="ExternalInput")
    msk = nc.dram_tensor("msk", [AH, B], f32, kind="ExternalInput")
    emX = nc.dram_tensor("emX", [AH, 3 * D], f32, kind="ExternalInput")
    ebm = nc.dram_tensor("ebm", [AH, 2 * KP + 2 * D], bf16, kind="ExternalInput")
    aux = nc.dram_tensor("aux", [AH, 2 * KP], f32, kind="ExternalInput")
    out = nc.dram_tensor("out", [1, 2], f32, kind="ExternalOutput")

    H = 256  # n-axis half width: the d2 chain is pipelined per half

    with tile.TileContext(nc) as tc:
        with (
            tc.tile_pool(name="sb", bufs=1) as sb,
            tc.tile_pool(name="junk", bufs=2) as junk,
            tc.tile_pool(name="jt", bufs=3) as jt,
            tc.tile_pool(name="psd", bufs=1, space="PSUM") as psd,
            tc.tile_pool(name="pst", bufs=1, space="PSUM") as pst,
            tc.tile_pool(name="pss", bufs=1, space="PSUM") as pss,
        ):
            emT_b = sb.tile([D, B], bf16)
            msk_t = sb.tile([AH, B], f32)
            emX_t = sb.tile([AH, 3 * D], f32)
            ebm_t = sb.tile([AH, 2 * KP + 2 * D], bf16)
            aux_t = sb.tile([AH, 2 * KP], f32)
            nc.sync.dma_start(out=emT_b[:], in_=emT[:])
            nc.scalar.dma_start(out=msk_t[:], in_=msk[:])
            nc.sync.dma_start(out=emX_t[:], in_=emX[:])
            nc.scalar.dma_start(out=ebm_t[:], in_=ebm[:])
            nc.sync.dma_start(out=aux_t[:], in_=aux[:])
            emN0_t = emX_t[:, 0:D]
            emN1_t = emX_t[:, D : 2 * D]
            emA_t = emX_t[:, 2 * D : 3 * D]
            E0_t = ebm_t[:, 0:KP]
            E1_t = ebm_t[:, KP : 2 * KP]
            mskT0_t = ebm_t[:, 2 * KP : 2 * KP + D]
            mskT1_t = ebm_t[:, 2 * KP + D : 2 * KP + 2 * D]
            vm_t = aux_t[:, 0:KP]
            exist_t = aux_t[:, KP : 2 * KP]

            ones_cb = sb.tile([D, 1], bf16)
            nc.vector.memset(ones_cb[:], 1.0)
            ones_rb = sb.tile([1, D], bf16)
            nc.vector.memset(ones_rb[:], 1.0)
            ones_cf = sb.tile([D, 1], f32)
            nc.vector.memset(ones_cf[:], 1.0)

            # ---- sq columns (DVE square-accumulate) and -2*emb prep
            em2n = sb.tile([D, H], bf16)
            nc.vector.tensor_scalar_mul(em2n[:], emT_b[:, 0:H], -2.0)
            emTAm2 = em2n[:, AOFF : AOFF + AH]
            sqn0 = sb.tile([AH, 1], f32)
            sqn1 = sb.tile([AH, 1], f32)
            sqa_sb = sb.tile([AH, 1], f32)
            for src, dst in ((emN0_t, sqn0), (emN1_t, sqn1), (emA_t, sqa_sb)):
                sj = junk.tile([AH, D], bf16, tag="sqj")
                nc.vector.scalar_tensor_tensor(
                    out=sj[:], in0=src, scalar=1.0, in1=src,
                    op0=OP.mult, op1=OP.mult, accum_out=dst,
                )
            e2b = sb.tile([D, B], bf16)
            nc.vector.tensor_mul(e2b[:], emT_b[:], emT_b[:])

            # ---- dotT chunks for the x-selection (no transposes)
            emTA_b = emT_b[:, AOFF : AOFF + AH]
            ps_dt0 = pst.tile([AH, AH], f32, tag="dt0")
            nc.tensor.matmul(ps_dt0[:], em2n[:, 0:AH], emTA_b, start=True, stop=True)
            ps_dt1 = pst.tile([AH, AH], f32, tag="dt1")
            nc.tensor.matmul(ps_dt1[:], em2n[:, AH:H], emTA_b, start=True, stop=True)

            # ---- d2 rows = -2 A^T E + sq[n], pipelined over two n-halves
            # (separate PSUM tiles per half so the halves don't serialize
            # on tile-granular WAR hazards). The dch -> ndneg chain is
            # emitted before the x-selection path: program order is the
            # scheduler's priority, and ndneg gates the main loop.
            ps_d2a = psd.tile([AH, H], f32, tag="d2a")
            ps_d2b = psd.tile([AH, H], f32, tag="d2b")
            ps_sq0 = pss.tile([1, H], f32, tag="sq0")
            ps_sq1 = pss.tile([1, H], f32, tag="sq1")
            ps_d2h = [ps_d2a, ps_d2b]
            ps_sqh = [ps_sq0, ps_sq1]
            sq_b = sb.tile([1, B], bf16)
            d2c = sb.tile([AH, B], f32)
            dch = sb.tile([AH, B], f32)
            ndneg_b = sb.tile([AH, B], bf16)
            with tc.high_priority():
                for h in range(2):
                    s = slice(h * H, (h + 1) * H)
                    pd = ps_d2h[h]
                    nc.tensor.matmul(
                        pd[:], emTAm2, emT_b[:, s], start=True, stop=False
                    )
                    nc.tensor.matmul(
                        ps_sqh[h][:], ones_cb[:], e2b[:, s], start=True, stop=True
                    )
                    nc.scalar.copy(sq_b[:, s], ps_sqh[h][:])
                    nc.tensor.matmul(
                        pd[:], ones_rb[:], sq_b[:, s], start=False, stop=True
                    )
                    nc.vector.tensor_scalar(
                        out=d2c[:, s], in0=pd[:], scalar1=sqa_sb[:], scalar2=0.0,
                        op0=OP.add, op1=OP.max,
                    )
                    nc.scalar.activation(dch[:, s], d2c[:, s], AF.Sqrt)
                    nc.vector.scalar_tensor_tensor(
                        out=ndneg_b[:, s], in0=msk_t[:, s], scalar=-BIG,
                        in1=dch[:, s], op0=OP.mult, op1=OP.subtract,
                    )

            # ---- x2[a,j]: d2mT chunks -> selection matmul -> fixup
            d2mT0 = sb.tile([AH, AH], bf16)
            nc.vector.scalar_tensor_tensor(
                out=d2mT0[:], in0=ps_dt0[:], scalar=sqn0[:], in1=mskT0_t,
                op0=OP.add, op1=OP.mult,
            )
            d2mT1 = sb.tile([AH, AH], bf16)
            nc.vector.scalar_tensor_tensor(
                out=d2mT1[:], in0=ps_dt1[:], scalar=sqn1[:], in1=mskT1_t,
                op0=OP.add, op1=OP.mult,
            )
            ps_xsel = pss.tile([AH, KP], f32, tag="xsel")
            nc.tensor.matmul(ps_xsel[:], d2mT0[:], E0_t, start=True, stop=False)
            nc.tensor.matmul(ps_xsel[:], d2mT1[:], E1_t, start=False, stop=True)
            x2a = sb.tile([AH, KP], f32)
            nc.vector.scalar_tensor_tensor(
                out=x2a[:], in0=exist_t, scalar=sqa_sb[:], in1=ps_xsel[:],
                op0=OP.mult, op1=OP.add,
            )
            x2b = sb.tile([AH, KP], f32)
            nc.vector.tensor_scalar_max(x2b[:], x2a[:], 0.0)
            xsq = sb.tile([AH, KP], f32)
            nc.scalar.activation(xsq[:], x2b[:], AF.Sqrt)
            xall = sb.tile([AH, KP], f32)
            nc.vector.scalar_tensor_tensor(
                out=xall[:], in0=xsq[:], scalar=MARGIN, in1=vm_t,
                op0=OP.add, op1=OP.mult,
            )
            xneg = sb.tile([AH, KP], f32)
            nc.vector.tensor_scalar_mul(xneg[:], xall[:], -1.0)

            # ---- main loop: ACT relu, PE row-accumulating matmul for the
            # relu sum, DVE+GpSimd is_gt fused-accum for the count
            cacc = sb.tile([AH, KP], f32)
            ps_r = pss.tile([1, B], f32, tag="pr")
            for j in range(KP):
                t = jt.tile([AH, B], bf16, tag="t")
                nc.scalar.activation(
                    t[:], ndneg_b[:], AF.Relu, bias=xall[:, j : j + 1], scale=1.0
                )
                nc.tensor.matmul(
                    ps_r[:], ones_cb[:], t[:], start=(j == 0), stop=(j == KP - 1)
                )
                g = junk.tile([AH, B], bf16, tag="g")
                nc.vector.tensor_scalar(
                    out=g[:], in0=ndneg_b[:], scalar1=xneg[:, j : j + 1],
                    scalar2=None, op0=OP.is_gt, op1=OP.add,
                    accum_out=cacc[:, j : j + 1],
                )

            # ---- final: count partition-matmul, relu-row reduce, one DMA
            ccol = sb.tile([AH, 1], f32)
            nc.vector.reduce_sum(ccol[:], cacc[:], axis=X)
            # reuse the long-dead ps_sq0 bank for the tiny count matmul
            nc.tensor.matmul(
                ps_sq0[0:1, 0:1], ones_cf[:], ccol[:], start=True, stop=True
            )
            res = sb.tile([1, 2], f32)
            nc.vector.reduce_sum(res[:, 0:1], ps_r[:], axis=X)
            nc.scalar.copy(res[:, 1:2], ps_sq0[0:1, 0:1])
            nc.sync.dma_start(out=out[:], in_=res[:])

    return nc


def _legalize_waits(bir: bytes) -> bytes:
    """walrus codegen in this toolchain allows only one sync-wait per
    instruction; split extra waits into standalone EventSemaphore insts."""
    import json

    m = json.loads(bir)
    for fn in m["functions"]:
        for bb in fn["blocks"]:
            new = []
            for inst in bb["instructions"]:
                si = inst.get("sync_info")
                if si and si.get("on_wait") and len(si["on_wait"]) > 1:
                    waits = si["on_wait"]
                    for j, w in enumerate(waits[:-1]):
                        new.append(
                            {
                                "engine": inst["engine"],
                                "ins": [],
                                "outs": [],
                                "name": f"{inst['name']}-w{j}",
                                "opcode": "EventSemaphore",
                                "sync_info": {"on_update": [], "on_wait": [w]},
                            }
                        )
                    si["on_wait"] = [waits[-1]]
                new.append(inst)
            bb["instructions"] = new
    return json.dumps(m).encode()


def _get_nc(KP):
    key = ("nc", KP)
    if key not in _CACHE:
        nc = _build_bass(KP)
        orig = nc.to_json_bytes
        nc.to_json_bytes = lambda: _legalize_waits(orig())
        _CACHE[key] = nc
    return _CACHE[key]


def _prep(idtys):
    """Host-side index work: stable sort by id, group geometry."""
    ids = np.asarray(idtys).astype(np.int64).reshape(B)
    order = np.argsort(ids, kind="stable")
    ids_sorted = ids[order]
    g_start = np.zeros(B, np.int64)
    g_size = np.zeros(B, np.int64)
    _, starts, counts = np.unique(ids_sorted, return_index=True, return_counts=True)
    for s, c in zip(starts, counts):
        g_start[s : s + c] = s
        g_size[s : s + c] = c
    rank_sorted = np.arange(B) - g_start
    smax = int(counts.max())
    return order, ids_sorted, rank_sorted, g_size, smax


def make_in_maps(embs: np.ndarray, idtys: np.ndarray):
    import ml_dtypes

    bf = ml_dtypes.bfloat16
    embs = np.ascontiguousarray(np.asarray(embs, dtype=np.float32))
    order, ids_sorted, rank_sorted, g_size, smax = _prep(idtys)
    KP = max((smax + 1) // 2, 1)
    idx = np.arange(B)
    in_maps = []
    for c in range(NCORES):
        b, par = c // 2, c % 2
        spos = (idx - AOFF + 128 * b) % B   # sorted position at rot position i
        rot = order[spos]                   # original sample at rot position i
        ids_rot = ids_sorted[spos]
        rank_rot = rank_sorted[spos]
        size_rot = g_size[spos]
        emT = np.ascontiguousarray(embs[rot].T.astype(bf))   # [D, B]
        a_sl = slice(AOFF, AOFF + AH)
        emN = embs[rot[0:256]]                                # [256, D]
        emA = embs[rot[a_sl]]                                 # [AH, D]
        emX = np.concatenate([emN[0:128], emN[128:256], emA], axis=1)
        mask = (ids_rot[a_sl][:, None] == ids_rot[None, :]).astype(np.float32)
        maskT = (ids_rot[0:256][:, None] == ids_rot[a_sl][None, :]).astype(bf)
        E = np.zeros((256, KP), np.float32)
        r256 = rank_rot[:256]
        sel = (r256 % 2 == par) & (r256 // 2 < KP)
        E[np.nonzero(sel)[0], r256[sel] // 2] = 1.0
        ra, sa = rank_rot[a_sl], size_rot[a_sl]
        rk = 2 * np.arange(KP)[None, :] + par               # [1, KP]
        vm = ((rk < sa[:, None]) & (rk != ra[:, None])).astype(np.float32)
        exist = (rk < sa[:, None]).astype(np.float32)
        ebm = np.concatenate(
            [E[:128].astype(bf), E[128:256].astype(bf), maskT[0:128], maskT[128:256]],
            axis=1,
        )
        auxm = np.concatenate([vm, exist], axis=1).astype(np.float32)
        in_maps.append(
            {
                "emT": emT,
                "msk": np.ascontiguousarray(mask),
                "emX": np.ascontiguousarray(emX.astype(np.float32)),
                "ebm": np.ascontiguousarray(ebm),
                "aux": np.ascontiguousarray(auxm),
            }
        )
    return in_maps, KP


def combine(results):
    total = 0.0
    count = 0.0
    for r in results:
        o = np.asarray(r["out"], dtype=np.float64)
        total += o[0, 0]
        count += o[0, 1]
    loss = np.float32(total / (count + 1e-16))
    return np.array(loss, dtype=np.float32)


def kernel(embs: np.ndarray, idtys: np.ndarray) -> np.ndarray:
    from concourse import bass_utils

    in_maps, KP = make_in_maps(np.asarray(embs), np.asarray(idtys))
    nc = _get_nc(KP)
    res = bass_utils.run_bass_kernel_spmd(nc, in_maps, list(range(NCORES)))
    return combine(res.results)


# revision 31
# speedup vs baseline: 1.1183x; 1.1183x over previous
"""BatchAllTripletLoss on 8 Trainium2 NeuronCores (sorted-layout version).

Contract: kernel(**inputs) takes the FULL inputs (embs [512,128] f32,
idtys [512] int64) and returns the FULL output (scalar f32 loss).

Math: d = pairwise euclidean distances [512,512];
  loss = sum_{a,p,n} relu(d[a,p]-d[a,n]+margin)*mask / (num_pos + eps)
The mask factorizes as pos[a,p]*neg[a,n] (pos: same id, p!=a; neg:
different id). All index work happens on the host: samples are sorted
by id so each group is contiguous, and per core the sample order is
rotated so its 128 anchors sit at positions AOFF..AOFF+127 and every
anchor's group lies inside positions [0, 256).

Per-core device program (core c: anchor block b=c//2, parity par=c%2,
handling positive ranks {par, par+2, ...} = KP columns):
 1. d2 rows for the 128 anchors via bf16 PE matmuls (-2*dot, ones@e2
    for sq[n], rank-1 sq[n] broadcast), pipelined over two n-halves;
    sq[a] via DVE square-accumulate; sq[a]-add + relu clamp fused in
    one DVE op; ACT sqrt -> d.
 2. ndneg = -(d + BIG*same) in bf16 (same-mask is a host input; BIG
    pushes same-id negatives out of every relu/count).
 3. x2[a,j] = d2[a, p_j(a)] selected in SQUARED space with NO
    transposes: dotT chunks [n,a] = -2 emb^T emb_A are plain matmuls,
    d2mT = (dotT + sq[n]) * maskT (host-transposed mask), one bf16
    matmul against a host rank-one-hot E, then sq[a]*exist fixup,
    clamp, tiny sqrt, margin/valid. No DRAM round trip, no indirect
    DMA, no device argsort.
 4. Main loop over KP rank columns, engines in parallel: ACT
    Relu(ndneg + x_j) -> t_j; PE ones-matmul accumulates the relu sum
    into PSUM [1,512]; DVE (ndneg > -x_j) fused-accum counts cols
    [0,336); GpSimd counts cols [336,512).
 5. Free-axis reduces -> out [128,3] = (cntA col, cntB col, relu sum
    in row 0); host sums columns across cores and divides.
"""

import numpy as np

B = 512
D = 128
NCORES = 8
AH = 128          # anchors per core
AOFF = 16         # rotated position of the first anchor
NSPL = 336        # count split: DVE does [0,NSPL), GpSimd the rest
MARGIN = 0.2
BIG = 1.0e6

_CACHE = {}


def _build_bass(KP):
    import concourse.bass as bass
    import concourse.tile as tile
    from concourse import mybir

    f32 = mybir.dt.float32
    bf16 = mybir.dt.bfloat16
    AF = mybir.ActivationFunctionType
    OP = mybir.AluOpType
    X = mybir.AxisListType.X

    nc = bass.Bass()

    emT = nc.dram_tensor("emT", [D, B], bf16, kind="ExternalInput")
    msk = nc.dram_tensor("msk", [AH, B], f32, kind="ExternalInput")
    emX = nc.dram_tensor("emX", [AH, 3 * D], f32, kind="ExternalInput")
    ebm = nc.dram_tensor("ebm", [AH, 2 * KP + 2 * D], bf16, kind="ExternalInput")
    aux = nc.dram_tensor("aux", [AH, 2 * KP], f32, kind="ExternalInput")
    out = nc.dram_tensor("out", [4, 2], f32, kind="ExternalOutput")

    H = 256  # n-axis half width: the d2 chain is pipelined per half

    with tile.TileContext(nc) as tc:
        with (
            tc.tile_pool(name="sb", bufs=1) as sb,
            tc.tile_pool(name="junk", bufs=2) as junk,
            tc.tile_pool(name="jt", bufs=3) as jt,
            tc.tile_pool(name="psd", bufs=1, space="PSUM") as psd,
            tc.tile_pool(name="pst", bufs=1, space="PSUM") as pst,
            tc.tile_pool(name="pss", bufs=1, space="PSUM") as pss,
        ):
            emT_b = sb.tile([D, B], bf16)
            msk_t = sb.tile([AH, B], f32)
            emX_t = sb.tile([AH, 3 * D], f32)
            ebm_t = sb.tile([AH, 2 * KP + 2 * D], bf16)
            aux_t = sb.tile([AH, 2 * KP], f32)
            nc.sync.dma_start(out=emT_b[:], in_=emT[:])
            nc.scalar.dma_start(out=msk_t[:], in_=msk[:])
            nc.sync.dma_start(out=emX_t[:], in_=emX[:])
            nc.scalar.dma_start(out=ebm_t[:], in_=ebm[:])
            nc.sync.dma_start(out=aux_t[:], in_=aux[:])
            emN0_t = emX_t[:, 0:D]
            emN1_t = emX_t[:, D : 2 * D]
            emA_t = emX_t[:, 2 * D : 3 * D]
            E0_t = ebm_t[:, 0:KP]
            E1_t = ebm_t[:, KP : 2 * KP]
            mskT0_t = ebm_t[:, 2 * KP : 2 * KP + D]
            mskT1_t = ebm_t[:, 2 * KP + D : 2 * KP + 2 * D]
            vm_t = aux_t[:, 0:KP]
            exist_t = aux_t[:, KP : 2 * KP]

            ones_cb = sb.tile([D, 1], bf16)
            nc.vector.memset(ones_cb[:], 1.0)
            ones_rb = sb.tile([1, D], bf16)
            nc.vector.memset(ones_rb[:], 1.0)
            cpad = sb.tile([AH, 32], f32)
            nc.vector.memset(cpad[:], 0.0)
            c4 = sb.tile([AH, 2], f32)
            nc.vector.memset(c4[:], 0.0)

            # ---- sq columns (DVE square-accumulate) and -2*emb prep
            em2n = sb.tile([D, H], bf16)
            nc.vector.tensor_scalar_mul(em2n[:], emT_b[:, 0:H], -2.0)
            emTAm2 = em2n[:, AOFF : AOFF + AH]
            sqn0 = sb.tile([AH, 1], f32)
            sqn1 = sb.tile([AH, 1], f32)
            sqa_sb = sb.tile([AH, 1], f32)
            for src, dst in ((emN0_t, sqn0), (emN1_t, sqn1), (emA_t, sqa_sb)):
                sj = junk.tile([AH, D], bf16, tag="sqj")
                nc.vector.scalar_tensor_tensor(
                    out=sj[:], in0=src, scalar=1.0, in1=src,
                    op0=OP.mult, op1=OP.mult, accum_out=dst,
                )
            e2b = sb.tile([D, B], bf16)
            nc.vector.tensor_mul(e2b[:], emT_b[:], emT_b[:])

            # ---- dotT chunks for the x-selection (no transposes)
            emTA_b = emT_b[:, AOFF : AOFF + AH]
            ps_dt0 = pst.tile([AH, AH], f32, tag="dt0")
            nc.tensor.matmul(ps_dt0[:], em2n[:, 0:AH], emTA_b, start=True, stop=True)
            ps_dt1 = pst.tile([AH, AH], f32, tag="dt1")
            nc.tensor.matmul(ps_dt1[:], em2n[:, AH:H], emTA_b, start=True, stop=True)

            # ---- d2 rows = -2 A^T E + sq[n], pipelined over two n-halves
            # (separate PSUM tiles per half so the halves don't serialize
            # on tile-granular WAR hazards). The dch -> ndneg chain is
            # emitted before the x-selection path: program order is the
            # scheduler's priority, and ndneg gates the main loop.
            ps_d2a = psd.tile([AH, H], f32, tag="d2a")
            ps_d2b = psd.tile([AH, H], f32, tag="d2b")
            ps_sq0 = pss.tile([1, H], f32, tag="sq0")
            ps_sq1 = pss.tile([1, H], f32, tag="sq1")
            ps_d2h = [ps_d2a, ps_d2b]
            ps_sqh = [ps_sq0, ps_sq1]
            sq_b = sb.tile([1, B], bf16)
            d2c = sb.tile([AH, B], f32)
            dch = sb.tile([AH, B], f32)
            ndneg_b = sb.tile([AH, B], bf16)
            with tc.high_priority():
                for h in range(2):
                    s = slice(h * H, (h + 1) * H)
                    pd = ps_d2h[h]
                    nc.tensor.matmul(
                        pd[:], emTAm2, emT_b[:, s], start=True, stop=False
                    )
                    nc.tensor.matmul(
                        ps_sqh[h][:], ones_cb[:], e2b[:, s], start=True, stop=True
                    )
                    nc.scalar.copy(sq_b[:, s], ps_sqh[h][:])
                    nc.tensor.matmul(
                        pd[:], ones_rb[:], sq_b[:, s], start=False, stop=True
                    )
                    nc.vector.tensor_scalar(
                        out=d2c[:, s], in0=pd[:], scalar1=sqa_sb[:], scalar2=0.0,
                        op0=OP.add, op1=OP.max,
                    )
                    nc.scalar.activation(dch[:, s], d2c[:, s], AF.Sqrt)
                    nc.vector.scalar_tensor_tensor(
                        out=ndneg_b[:, s], in0=msk_t[:, s], scalar=-BIG,
                        in1=dch[:, s], op0=OP.mult, op1=OP.subtract,
                    )

            # ---- x2[a,j]: d2mT chunks -> selection matmul -> fixup
            d2mT0 = sb.tile([AH, AH], bf16)
            nc.vector.scalar_tensor_tensor(
                out=d2mT0[:], in0=ps_dt0[:], scalar=sqn0[:], in1=mskT0_t,
                op0=OP.add, op1=OP.mult,
            )
            d2mT1 = sb.tile([AH, AH], bf16)
            nc.vector.scalar_tensor_tensor(
                out=d2mT1[:], in0=ps_dt1[:], scalar=sqn1[:], in1=mskT1_t,
                op0=OP.add, op1=OP.mult,
            )
            ps_xsel = pss.tile([AH, KP], f32, tag="xsel")
            nc.tensor.matmul(ps_xsel[:], d2mT0[:], E0_t, start=True, stop=False)
            nc.tensor.matmul(ps_xsel[:], d2mT1[:], E1_t, start=False, stop=True)
            x2a = sb.tile([AH, KP], f32)
            nc.vector.scalar_tensor_tensor(
                out=x2a[:], in0=exist_t, scalar=sqa_sb[:], in1=ps_xsel[:],
                op0=OP.mult, op1=OP.add,
            )
            x2b = sb.tile([AH, KP], f32)
            nc.vector.tensor_scalar_max(x2b[:], x2a[:], 0.0)
            xsq = sb.tile([AH, KP], f32)
            nc.scalar.activation(xsq[:], x2b[:], AF.Sqrt)
            xall = sb.tile([AH, KP], f32)
            nc.vector.scalar_tensor_tensor(
                out=xall[:], in0=xsq[:], scalar=MARGIN, in1=vm_t,
                op0=OP.add, op1=OP.mult,
            )
            xneg = sb.tile([AH, KP], f32)
            nc.vector.tensor_scalar_mul(xneg[:], xall[:], -1.0)

            # ---- main loop: ACT relu, PE row-accumulating matmul for the
            # relu sum, DVE+GpSimd is_gt fused-accum for the count
            cacc = sb.tile([AH, KP], f32)
            ps_r = pss.tile([1, B], f32, tag="pr")
            for j in range(KP):
                t = jt.tile([AH, B], bf16, tag="t")
                nc.scalar.activation(
                    t[:], ndneg_b[:], AF.Relu, bias=xall[:, j : j + 1], scale=1.0
                )
                nc.tensor.matmul(
                    ps_r[:], ones_cb[:], t[:], start=(j == 0), stop=(j == KP - 1)
                )
                g = junk.tile([AH, B], bf16, tag="g")
                nc.vector.tensor_scalar(
                    out=g[:], in0=ndneg_b[:], scalar1=xneg[:, j : j + 1],
                    scalar2=None, op0=OP.is_gt, op1=OP.add,
                    accum_out=cacc[:, j : j + 1],
                )

            # ---- final: all-DVE count fold via 32x32 block transpose
            # (no PE work after the loop -- the static scheduler would
            # hoist it between loop matmuls and stall the PE queue).
            # ccol -> cpad col 0; block transpose spreads each 32-row
            # group onto one partition row {0,32,64,96}; free-reduce
            # leaves 4 partials; a partition-strided DMA emits them.
            nc.vector.reduce_sum(cpad[:, 0:1], cacc[:], axis=X)
            ctr = sb.tile([AH, 32], f32)
            nc.vector.transpose(ctr[:], cpad[:])
            nc.vector.reduce_sum(c4[:, 0:1], ctr[:], axis=X)
            nc.vector.reduce_sum(c4[0:1, 1:2], ps_r[:], axis=X)
            nc.sync.dma_start(out=out[:], in_=c4[0:128:32, 0:2])

    return nc


def _legalize_waits(bir: bytes) -> bytes:
    """walrus codegen in this toolchain allows only one sync-wait per
    instruction; split extra waits into standalone EventSemaphore insts."""
    import json

    m = json.loads(bir)
    for fn in m["functions"]:
        for bb in fn["blocks"]:
            new = []
            for inst in bb["instructions"]:
                si = inst.get("sync_info")
                if si and si.get("on_wait") and len(si["on_wait"]) > 1:
                    waits = si["on_wait"]
                    for j, w in enumerate(waits[:-1]):
                        new.append(
                            {
                                "engine": inst["engine"],
                                "ins": [],
                                "outs": [],
                                "name": f"{inst['name']}-w{j}",
                                "opcode": "EventSemaphore",
                                "sync_info": {"on_update": [], "on_wait": [w]},
                            }
                        )
                    si["on_wait"] = [waits[-1]]
                new.append(inst)
            bb["instructions"] = new
    return json.dumps(m).encode()


def _get_nc(KP):
    key = ("nc", KP)
    if key not in _CACHE:
        nc = _build_bass(KP)
        orig = nc.to_json_bytes
        nc.to_json_bytes = lambda: _legalize_waits(orig())
        _CACHE[key] = nc
    return _CACHE[key]


def _prep(idtys):
    """Host-side index work: stable sort by id, group geometry."""
    ids = np.asarray(idtys).astype(np.int64).reshape(B)
    order = np.argsort(ids, kind="stable")
    ids_sorted = ids[order]
    g_start = np.zeros(B, np.int64)
    g_size = np.zeros(B, np.int64)
    _, starts, counts = np.unique(ids_sorted, return_index=True, return_counts=True)
    for s, c in zip(starts, counts):
        g_start[s : s + c] = s
        g_size[s : s + c] = c
    rank_sorted = np.arange(B) - g_start
    smax = int(counts.max())
    return order, ids_sorted, rank_sorted, g_size, smax


def make_in_maps(embs: np.ndarray, idtys: np.ndarray):
    import ml_dtypes

    bf = ml_dtypes.bfloat16
    embs = np.ascontiguousarray(np.asarray(embs, dtype=np.float32))
    order, ids_sorted, rank_sorted, g_size, smax = _prep(idtys)
    KP = max((smax + 1) // 2, 1)
    idx = np.arange(B)
    in_maps = []
    for c in range(NCORES):
        b, par = c // 2, c % 2
        spos = (idx - AOFF + 128 * b) % B   # sorted position at rot position i
        rot = order[spos]                   # original sample at rot position i
        ids_rot = ids_sorted[spos]
        rank_rot = rank_sorted[spos]
        size_rot = g_size[spos]
        emT = np.ascontiguousarray(embs[rot].T.astype(bf))   # [D, B]
        a_sl = slice(AOFF, AOFF + AH)
        emN = embs[rot[0:256]]                                # [256, D]
        emA = embs[rot[a_sl]]                                 # [AH, D]
        emX = np.concatenate([emN[0:128], emN[128:256], emA], axis=1)
        mask = (ids_rot[a_sl][:, None] == ids_rot[None, :]).astype(np.float32)
        maskT = (ids_rot[0:256][:, None] == ids_rot[a_sl][None, :]).astype(bf)
        E = np.zeros((256, KP), np.float32)
        r256 = rank_rot[:256]
        sel = (r256 % 2 == par) & (r256 // 2 < KP)
        E[np.nonzero(sel)[0], r256[sel] // 2] = 1.0
        ra, sa = rank_rot[a_sl], size_rot[a_sl]
        rk = 2 * np.arange(KP)[None, :] + par               # [1, KP]
        vm = ((rk < sa[:, None]) & (rk != ra[:, None])).astype(np.float32)
        exist = (rk < sa[:, None]).astype(np.float32)
        ebm = np.concatenate(
            [E[:128].astype(bf), E[128:256].astype(bf), maskT[0:128], maskT[128:256]],
            axis=1,
        )
        auxm = np.concatenate([vm, exist], axis=1).astype(np.float32)
        in_maps.append(
            {
                "emT": emT,
                "msk": np.ascontiguousarray(mask),
                "emX": np.ascontiguousarray(emX.astype(np.float32)),
                "ebm": np.ascontiguousarray(ebm),
                "aux": np.ascontiguousarray(auxm),
            }
        )
    return in_maps, KP


def combine(results):
    total = 0.0
    count = 0.0
    for r in results:
        o = np.asarray(r["out"], dtype=np.float64)
        total += o[0, 1]
        count += o[:, 0].sum()
    loss = np.float32(total / (count + 1e-16))
    return np.array(loss, dtype=np.float32)


def kernel(embs: np.ndarray, idtys: np.ndarray) -> np.ndarray:
    from concourse import bass_utils

    in_maps, KP = make_in_maps(np.asarray(embs), np.asarray(idtys))
    nc = _get_nc(KP)
    res = bass_utils.run_bass_kernel_spmd(nc, in_maps, list(range(NCORES)))
    return combine(res.results)


# revision 33
# speedup vs baseline: 1.1192x; 1.0008x over previous
"""BatchAllTripletLoss on 8 Trainium2 NeuronCores (sorted-layout version).

Contract: kernel(**inputs) takes the FULL inputs (embs [512,128] f32,
idtys [512] int64) and returns the FULL output (scalar f32 loss).

Math: d = pairwise euclidean distances [512,512];
  loss = sum_{a,p,n} relu(d[a,p]-d[a,n]+margin)*mask / (num_pos + eps)
The mask factorizes as pos[a,p]*neg[a,n] (pos: same id, p!=a; neg:
different id). All index work happens on the host: samples are sorted
by id so each group is contiguous, and per core the sample order is
rotated so its 128 anchors sit at positions AOFF..AOFF+127 and every
anchor's group lies inside positions [0, 256).

Per-core device program (core c: anchor block b=c//2, parity par=c%2,
handling positive ranks {par, par+2, ...} = KP columns):
 1. d2 rows for the 128 anchors via bf16 PE matmuls (-2*dot, ones@e2
    for sq[n], rank-1 sq[n] broadcast), pipelined over two n-halves;
    sq[a] via DVE square-accumulate; sq[a]-add + relu clamp fused in
    one DVE op; ACT sqrt -> d.
 2. ndneg = -(d + BIG*same) in bf16 (same-mask is a host input; BIG
    pushes same-id negatives out of every relu/count).
 3. x2[a,j] = d2[a, p_j(a)] selected in SQUARED space with NO
    transposes: dotT chunks [n,a] = -2 emb^T emb_A are plain matmuls,
    d2mT = (dotT + sq[n]) * maskT (host-transposed mask), one bf16
    matmul against a host rank-one-hot E, then sq[a]*exist fixup,
    clamp, tiny sqrt, margin/valid. No DRAM round trip, no indirect
    DMA, no device argsort.
 4. Main loop over KP rank columns, engines in parallel: ACT
    Relu(ndneg + x_j) -> t_j; PE ones-matmul accumulates the relu sum
    into PSUM [1,512]; DVE (ndneg > -x_j) fused-accum counts cols
    [0,336); GpSimd counts cols [336,512).
 5. Free-axis reduces -> out [128,3] = (cntA col, cntB col, relu sum
    in row 0); host sums columns across cores and divides.
"""

import numpy as np

B = 512
D = 128
NCORES = 8
AH = 128          # anchors per core
AOFF = 16         # rotated position of the first anchor
NSPL = 336        # count split: DVE does [0,NSPL), GpSimd the rest
MARGIN = 0.2
BIG = 1.0e6

_CACHE = {}


def _build_bass(KP):
    import concourse.bass as bass
    import concourse.tile as tile
    from concourse import mybir

    f32 = mybir.dt.float32
    bf16 = mybir.dt.bfloat16
    AF = mybir.ActivationFunctionType
    OP = mybir.AluOpType
    X = mybir.AxisListType.X

    nc = bass.Bass()

    emT = nc.dram_tensor("emT", [D, B], bf16, kind="ExternalInput")
    msk = nc.dram_tensor("msk", [AH, B], f32, kind="ExternalInput")
    emX = nc.dram_tensor("emX", [AH, 3 * D], f32, kind="ExternalInput")
    ebm = nc.dram_tensor("ebm", [AH, 2 * KP + 2 * D], bf16, kind="ExternalInput")
    aux = nc.dram_tensor("aux", [AH, 2 * KP], f32, kind="ExternalInput")
    out = nc.dram_tensor("out", [4, 2], f32, kind="ExternalOutput")

    H = 256  # n-axis half width: the d2 chain is pipelined per half

    with tile.TileContext(nc) as tc:
        with (
            tc.tile_pool(name="sb", bufs=1) as sb,
            tc.tile_pool(name="junk", bufs=2) as junk,
            tc.tile_pool(name="jt", bufs=3) as jt,
            tc.tile_pool(name="psd", bufs=1, space="PSUM") as psd,
            tc.tile_pool(name="pst", bufs=1, space="PSUM") as pst,
            tc.tile_pool(name="pss", bufs=1, space="PSUM") as pss,
        ):
            emT_b = sb.tile([D, B], bf16)
            msk_t = sb.tile([AH, B], f32)
            emX_t = sb.tile([AH, 3 * D], f32)
            ebm_t = sb.tile([AH, 2 * KP + 2 * D], bf16)
            aux_t = sb.tile([AH, 2 * KP], f32)
            nc.sync.dma_start(out=emT_b[:], in_=emT[:])
            nc.scalar.dma_start(out=msk_t[:], in_=msk[:])
            nc.sync.dma_start(out=emX_t[:], in_=emX[:])
            nc.scalar.dma_start(out=ebm_t[:], in_=ebm[:])
            nc.sync.dma_start(out=aux_t[:], in_=aux[:])
            emN0_t = emX_t[:, 0:D]
            emN1_t = emX_t[:, D : 2 * D]
            emA_t = emX_t[:, 2 * D : 3 * D]
            E0_t = ebm_t[:, 0:KP]
            E1_t = ebm_t[:, KP : 2 * KP]
            mskT0_t = ebm_t[:, 2 * KP : 2 * KP + D]
            mskT1_t = ebm_t[:, 2 * KP + D : 2 * KP + 2 * D]
            vm_t = aux_t[:, 0:KP]
            exist_t = aux_t[:, KP : 2 * KP]

            ones_cb = sb.tile([D, 1], bf16)
            nc.vector.memset(ones_cb[:], 1.0)
            ones_rb = sb.tile([1, D], bf16)
            nc.vector.memset(ones_rb[:], 1.0)
            cpad = sb.tile([AH, 32], f32)
            nc.vector.memset(cpad[:], 0.0)
            c4 = sb.tile([AH, 2], f32)
            nc.vector.memset(c4[:], 0.0)

            # ---- sq columns (DVE square-accumulate) and -2*emb prep
            em2n = sb.tile([D, H], bf16)
            nc.vector.tensor_scalar_mul(em2n[:], emT_b[:, 0:H], -2.0)
            emTAm2 = em2n[:, AOFF : AOFF + AH]
            sqn0 = sb.tile([AH, 1], f32)
            sqn1 = sb.tile([AH, 1], f32)
            sqa_sb = sb.tile([AH, 1], f32)
            for src, dst in ((emN0_t, sqn0), (emN1_t, sqn1), (emA_t, sqa_sb)):
                sj = junk.tile([AH, D], bf16, tag="sqj")
                nc.vector.scalar_tensor_tensor(
                    out=sj[:], in0=src, scalar=1.0, in1=src,
                    op0=OP.mult, op1=OP.mult, accum_out=dst,
                )
            e2b = sb.tile([D, B], bf16)
            nc.vector.tensor_mul(e2b[:], emT_b[:], emT_b[:])

            # ---- dotT chunks for the x-selection (no transposes)
            emTA_b = emT_b[:, AOFF : AOFF + AH]
            ps_dt0 = pst.tile([AH, AH], f32, tag="dt0")
            nc.tensor.matmul(ps_dt0[:], em2n[:, 0:AH], emTA_b, start=True, stop=True)
            ps_dt1 = pst.tile([AH, AH], f32, tag="dt1")
            nc.tensor.matmul(ps_dt1[:], em2n[:, AH:H], emTA_b, start=True, stop=True)

            # ---- d2 rows = -2 A^T E + sq[n], pipelined over two n-halves
            # (separate PSUM tiles per half so the halves don't serialize
            # on tile-granular WAR hazards). The dch -> ndneg chain is
            # emitted before the x-selection path: program order is the
            # scheduler's priority, and ndneg gates the main loop.
            ps_d2a = psd.tile([AH, H], f32, tag="d2a")
            ps_d2b = psd.tile([AH, H], f32, tag="d2b")
            ps_sq0 = pss.tile([1, H], f32, tag="sq0")
            ps_sq1 = pss.tile([1, H], f32, tag="sq1")
            ps_d2h = [ps_d2a, ps_d2b]
            ps_sqh = [ps_sq0, ps_sq1]
            sq_b = sb.tile([1, B], bf16)
            d2c = sb.tile([AH, B], f32)
            dch = sb.tile([AH, B], f32)
            ndneg_b = sb.tile([AH, B], bf16)
            r1_insts = []
            with tc.high_priority():
                for h in range(2):
                    s = slice(h * H, (h + 1) * H)
                    pd = ps_d2h[h]
                    nc.tensor.matmul(
                        pd[:], emTAm2, emT_b[:, s], start=True, stop=False
                    )
                    nc.tensor.matmul(
                        ps_sqh[h][:], ones_cb[:], e2b[:, s], start=True, stop=True
                    )
                    nc.scalar.copy(sq_b[:, s], ps_sqh[h][:])
                    r1_insts.append(
                        nc.tensor.matmul(
                            pd[:], ones_rb[:], sq_b[:, s], start=False, stop=True
                        )
                    )
                    nc.vector.tensor_scalar(
                        out=d2c[:, s], in0=pd[:], scalar1=sqa_sb[:], scalar2=0.0,
                        op0=OP.add, op1=OP.max,
                    )
                    nc.scalar.activation(dch[:, s], d2c[:, s], AF.Sqrt)
                    nc.vector.scalar_tensor_tensor(
                        out=ndneg_b[:, s], in0=msk_t[:, s], scalar=-BIG,
                        in1=dch[:, s], op0=OP.mult, op1=OP.subtract,
                    )

            # ---- x2[a,j]: d2mT chunks -> selection matmul -> fixup
            d2mT0 = sb.tile([AH, AH], bf16)
            nc.vector.scalar_tensor_tensor(
                out=d2mT0[:], in0=ps_dt0[:], scalar=sqn0[:], in1=mskT0_t,
                op0=OP.add, op1=OP.mult,
            )
            d2mT1 = sb.tile([AH, AH], bf16)
            nc.vector.scalar_tensor_tensor(
                out=d2mT1[:], in0=ps_dt1[:], scalar=sqn1[:], in1=mskT1_t,
                op0=OP.add, op1=OP.mult,
            )
            ps_xsel = pss.tile([AH, KP], f32, tag="xsel")
            xm0 = nc.tensor.matmul(ps_xsel[:], d2mT0[:], E0_t, start=True, stop=False)
            nc.tensor.matmul(ps_xsel[:], d2mT1[:], E1_t, start=False, stop=True)
            # keep the PE queue on the critical d2 chain: the selection
            # matmuls have ~2us of slack, the rank-1s gate the main loop
            bass._add_dep_helper(
                xm0.ins, r1_insts[-1].ins, sync=True, reason="pe order"
            )
            x2a = sb.tile([AH, KP], f32)
            nc.vector.scalar_tensor_tensor(
                out=x2a[:], in0=exist_t, scalar=sqa_sb[:], in1=ps_xsel[:],
                op0=OP.mult, op1=OP.add,
            )
            x2b = sb.tile([AH, KP], f32)
            nc.vector.tensor_scalar_max(x2b[:], x2a[:], 0.0)
            xsq = sb.tile([AH, KP], f32)
            nc.scalar.activation(xsq[:], x2b[:], AF.Sqrt)
            xall = sb.tile([AH, KP], f32)
            nc.vector.scalar_tensor_tensor(
                out=xall[:], in0=xsq[:], scalar=MARGIN, in1=vm_t,
                op0=OP.add, op1=OP.mult,
            )
            xneg = sb.tile([AH, KP], f32)
            nc.vector.tensor_scalar_mul(xneg[:], xall[:], -1.0)

            # ---- main loop: ACT relu, PE row-accumulating matmul for the
            # relu sum, DVE+GpSimd is_gt fused-accum for the count
            cacc = sb.tile([AH, KP], f32)
            ps_r = pss.tile([1, B], f32, tag="pr")
            for j in range(KP):
                t = jt.tile([AH, B], bf16, tag="t")
                nc.scalar.activation(
                    t[:], ndneg_b[:], AF.Relu, bias=xall[:, j : j + 1], scale=1.0
                )
                nc.tensor.matmul(
                    ps_r[:], ones_cb[:], t[:], start=(j == 0), stop=(j == KP - 1)
                )
                g = junk.tile([AH, B], bf16, tag="g")
                nc.vector.tensor_scalar(
                    out=g[:], in0=ndneg_b[:], scalar1=xneg[:, j : j + 1],
                    scalar2=None, op0=OP.is_gt, op1=OP.add,
                    accum_out=cacc[:, j : j + 1],
                )

            # ---- final: all-DVE count fold via 32x32 block transpose
            # (no PE work after the loop -- the static scheduler would
            # hoist it between loop matmuls and stall the PE queue).
            # ccol -> cpad col 0; block transpose spreads each 32-row
            # group onto one partition row {0,32,64,96}; free-reduce
            # leaves 4 partials; a partition-strided DMA emits them.
            nc.vector.reduce_sum(cpad[:, 0:1], cacc[:], axis=X)
            ctr = sb.tile([AH, 32], f32)
            nc.vector.transpose(ctr[:], cpad[:])
            nc.vector.reduce_sum(c4[:, 0:1], ctr[:], axis=X)
            nc.vector.reduce_sum(c4[0:1, 1:2], ps_r[:], axis=X)
            nc.sync.dma_start(out=out[:], in_=c4[0:128:32, 0:2])

    return nc


def _legalize_waits(bir: bytes) -> bytes:
    """walrus codegen in this toolchain allows only one sync-wait per
    instruction; split extra waits into standalone EventSemaphore insts."""
    import json

    m = json.loads(bir)
    for fn in m["functions"]:
        for bb in fn["blocks"]:
            new = []
            for inst in bb["instructions"]:
                si = inst.get("sync_info")
                if si and si.get("on_wait") and len(si["on_wait"]) > 1:
                    waits = si["on_wait"]
                    for j, w in enumerate(waits[:-1]):
                        new.append(
                            {
                                "engine": inst["engine"],
                                "ins": [],
                                "outs": [],
                                "name": f"{inst['name']}-w{j}",
                                "opcode": "EventSemaphore",
                                "sync_info": {"on_update": [], "on_wait": [w]},
                            }
                        )
                    si["on_wait"] = [waits[-1]]
                new.append(inst)
            bb["instructions"] = new
    return json.dumps(m).encode()


def _get_nc(KP):
    key = ("nc", KP)
    if key not in _CACHE:
        nc = _build_bass(KP)
        orig = nc.to_json_bytes
        nc.to_json_bytes = lambda: _legalize_waits(orig())
        _CACHE[key] = nc
    return _CACHE[key]


def _prep(idtys):
    """Host-side index work: stable sort by id, group geometry."""
    ids = np.asarray(idtys).astype(np.int64).reshape(B)
    order = np.argsort(ids, kind="stable")
    ids_sorted = ids[order]
    g_start = np.zeros(B, np.int64)
    g_size = np.zeros(B, np.int64)
    _, starts, counts = np.unique(ids_sorted, return_index=True, return_counts=True)
    for s, c in zip(starts, counts):
        g_start[s : s + c] = s
        g_size[s : s + c] = c
    rank_sorted = np.arange(B) - g_start
    smax = int(counts.max())
    return order, ids_sorted, rank_sorted, g_size, smax


def make_in_maps(embs: np.ndarray, idtys: np.ndarray):
    import ml_dtypes

    bf = ml_dtypes.bfloat16
    embs = np.ascontiguousarray(np.asarray(embs, dtype=np.float32))
    order, ids_sorted, rank_sorted, g_size, smax = _prep(idtys)
    KP = max((smax + 1) // 2, 1)
    idx = np.arange(B)
    in_maps = []
    for c in range(NCORES):
        b, par = c // 2, c % 2
        spos = (idx - AOFF + 128 * b) % B   # sorted position at rot position i
        rot = order[spos]                   # original sample at rot position i
        ids_rot = ids_sorted[spos]
        rank_rot = rank_sorted[spos]
        size_rot = g_size[spos]
        emT = np.ascontiguousarray(embs[rot].T.astype(bf))   # [D, B]
        a_sl = slice(AOFF, AOFF + AH)
        emN = embs[rot[0:256]]                                # [256, D]
        emA = embs[rot[a_sl]]                                 # [AH, D]
        emX = np.concatenate([emN[0:128], emN[128:256], emA], axis=1)
        mask = (ids_rot[a_sl][:, None] == ids_rot[None, :]).astype(np.float32)
        maskT = (ids_rot[0:256][:, None] == ids_rot[a_sl][None, :]).astype(bf)
        E = np.zeros((256, KP), np.float32)
        r256 = rank_rot[:256]
        sel = (r256 % 2 == par) & (r256 // 2 < KP)
        E[np.nonzero(sel)[0], r256[sel] // 2] = 1.0
        ra, sa = rank_rot[a_sl], size_rot[a_sl]
        rk = 2 * np.arange(KP)[None, :] + par               # [1, KP]
        vm = ((rk < sa[:, None]) & (rk != ra[:, None])).astype(np.float32)
        exist = (rk < sa[:, None]).astype(np.float32)
        ebm = np.concatenate(
            [E[:128].astype(bf), E[128:256].astype(bf), maskT[0:128], maskT[128:256]],
            axis=1,
        )
        auxm = np.concatenate([vm, exist], axis=1).astype(np.float32)
        in_maps.append(
            {
                "emT": emT,
                "msk": np.ascontiguousarray(mask),
                "emX": np.ascontiguousarray(emX.astype(np.float32)),
                "ebm": np.ascontiguousarray(ebm),
                "aux": np.ascontiguousarray(auxm),
            }
        )
    return in_maps, KP


def combine(results):
    total = 0.0
    count = 0.0
    for r in results:
        o = np.asarray(r["out"], dtype=np.float64)
        total += o[0, 1]
        count += o[:, 0].sum()
    loss = np.float32(total / (count + 1e-16))
    return np.array(loss, dtype=np.float32)


def kernel(embs: np.ndarray, idtys: np.ndarray) -> np.ndarray:
    from concourse import bass_utils

    in_maps, KP = make_in_maps(np.asarray(embs), np.asarray(idtys))
    nc = _get_nc(KP)
    res = bass_utils.run_bass_kernel_spmd(nc, in_maps, list(range(NCORES)))
    return combine(res.results)


# revision 34
# speedup vs baseline: 1.1343x; 1.0135x over previous
"""BatchAllTripletLoss on 8 Trainium2 NeuronCores (sorted-layout version).

Contract: kernel(**inputs) takes the FULL inputs (embs [512,128] f32,
idtys [512] int64) and returns the FULL output (scalar f32 loss).

Math: d = pairwise euclidean distances [512,512];
  loss = sum_{a,p,n} relu(d[a,p]-d[a,n]+margin)*mask / (num_pos + eps)
The mask factorizes as pos[a,p]*neg[a,n] (pos: same id, p!=a; neg:
different id). All index work happens on the host: samples are sorted
by id so each group is contiguous, and per core the sample order is
rotated so its 128 anchors sit at positions AOFF..AOFF+127 and every
anchor's group lies inside positions [0, 256).

Per-core device program (core c: anchor block b=c//2, parity par=c%2,
handling positive ranks {par, par+2, ...} = KP columns):
 1. d2 rows for the 128 anchors via bf16 PE matmuls (-2*dot, ones@e2
    for sq[n], rank-1 sq[n] broadcast), pipelined over two n-halves;
    sq[a] via DVE square-accumulate; sq[a]-add + relu clamp fused in
    one DVE op; ACT sqrt -> d.
 2. ndneg = -(d + BIG*same) in bf16 (same-mask is a host input; BIG
    pushes same-id negatives out of every relu/count).
 3. x2[a,j] = d2[a, p_j(a)] selected in SQUARED space with NO
    transposes: dotT chunks [n,a] = -2 emb^T emb_A are plain matmuls,
    d2mT = (dotT + sq[n]) * maskT (host-transposed mask), one bf16
    matmul against a host rank-one-hot E, then sq[a]*exist fixup,
    clamp, tiny sqrt, margin/valid. No DRAM round trip, no indirect
    DMA, no device argsort.
 4. Main loop over KP rank columns, engines in parallel: ACT
    Relu(ndneg + x_j) -> t_j; PE ones-matmul accumulates the relu sum
    into PSUM [1,512]; DVE (ndneg > -x_j) fused-accum counts cols
    [0,336); GpSimd counts cols [336,512).
 5. Free-axis reduces -> out [128,3] = (cntA col, cntB col, relu sum
    in row 0); host sums columns across cores and divides.
"""

import numpy as np

B = 512
D = 128
NCORES = 8
AH = 128          # anchors per core
AOFF = 16         # rotated position of the first anchor
NSPL = 336        # count split: DVE does [0,NSPL), GpSimd the rest
MARGIN = 0.2
BIG = 1.0e6

_CACHE = {}


def _build_bass(KP):
    import concourse.bass as bass
    import concourse.tile as tile
    from concourse import mybir

    f32 = mybir.dt.float32
    bf16 = mybir.dt.bfloat16
    AF = mybir.ActivationFunctionType
    OP = mybir.AluOpType
    X = mybir.AxisListType.X

    nc = bass.Bass()

    emT = nc.dram_tensor("emT", [D, B], bf16, kind="ExternalInput")
    msk = nc.dram_tensor("msk", [AH, B], f32, kind="ExternalInput")
    emX = nc.dram_tensor("emX", [AH, 3 * D], f32, kind="ExternalInput")
    ebm = nc.dram_tensor("ebm", [AH, 2 * KP + 2 * D], bf16, kind="ExternalInput")
    aux = nc.dram_tensor("aux", [AH, 2 * KP], f32, kind="ExternalInput")
    out = nc.dram_tensor("out", [4, 2], f32, kind="ExternalOutput")

    H = 256  # n-axis half width: the d2 chain is pipelined per half

    with tile.TileContext(nc) as tc:
        with (
            tc.tile_pool(name="sb", bufs=1) as sb,
            tc.tile_pool(name="junk", bufs=2) as junk,
            tc.tile_pool(name="jt", bufs=3) as jt,
            tc.tile_pool(name="psd", bufs=1, space="PSUM") as psd,
            tc.tile_pool(name="pst", bufs=1, space="PSUM") as pst,
            tc.tile_pool(name="pss", bufs=1, space="PSUM") as pss,
        ):
            emT_b = sb.tile([D, B], bf16)
            msk_t = sb.tile([AH, B], f32)
            emX_t = sb.tile([AH, 3 * D], f32)
            ebm_t = sb.tile([AH, 2 * KP + 2 * D], bf16)
            aux_t = sb.tile([AH, 2 * KP], f32)
            nc.sync.dma_start(out=emT_b[:], in_=emT[:])
            nc.scalar.dma_start(out=msk_t[:], in_=msk[:])
            nc.sync.dma_start(out=emX_t[:], in_=emX[:])
            nc.scalar.dma_start(out=ebm_t[:], in_=ebm[:])
            nc.sync.dma_start(out=aux_t[:], in_=aux[:])
            emN0_t = emX_t[:, 0:D]
            emN1_t = emX_t[:, D : 2 * D]
            emA_t = emX_t[:, 2 * D : 3 * D]
            E0_t = ebm_t[:, 0:KP]
            E1_t = ebm_t[:, KP : 2 * KP]
            mskT0_t = ebm_t[:, 2 * KP : 2 * KP + D]
            mskT1_t = ebm_t[:, 2 * KP + D : 2 * KP + 2 * D]
            vm_t = aux_t[:, 0:KP]
            exist_t = aux_t[:, KP : 2 * KP]

            ones_cb = sb.tile([D, 1], bf16)
            nc.vector.memset(ones_cb[:], 1.0)
            ones_rb = sb.tile([1, D], bf16)
            nc.vector.memset(ones_rb[:], 1.0)
            cpad = sb.tile([AH, 32], f32)
            nc.vector.memset(cpad[:], 0.0)
            c4 = sb.tile([AH, 2], f32)
            nc.vector.memset(c4[:], 0.0)

            # ---- sq columns (DVE square-accumulate) and -2*emb prep
            em2n = sb.tile([D, H], bf16)
            nc.vector.tensor_scalar_mul(em2n[:], emT_b[:, 0:H], -2.0)
            emTAm2 = em2n[:, AOFF : AOFF + AH]
            sqn0 = sb.tile([AH, 1], f32)
            sqn1 = sb.tile([AH, 1], f32)
            sqa_sb = sb.tile([AH, 1], f32)
            for src, dst in ((emN0_t, sqn0), (emN1_t, sqn1), (emA_t, sqa_sb)):
                sj = junk.tile([AH, D], bf16, tag="sqj")
                nc.vector.scalar_tensor_tensor(
                    out=sj[:], in0=src, scalar=1.0, in1=src,
                    op0=OP.mult, op1=OP.mult, accum_out=dst,
                )
            e2b = sb.tile([D, B], bf16)
            nc.vector.tensor_mul(e2b[:], emT_b[:], emT_b[:])

            # ---- dotT chunks for the x-selection (no transposes)
            emTA_b = emT_b[:, AOFF : AOFF + AH]
            ps_dt0 = pst.tile([AH, AH], f32, tag="dt0")
            nc.tensor.matmul(ps_dt0[:], em2n[:, 0:AH], emTA_b, start=True, stop=True)
            ps_dt1 = pst.tile([AH, AH], f32, tag="dt1")
            nc.tensor.matmul(ps_dt1[:], em2n[:, AH:H], emTA_b, start=True, stop=True)

            # ---- d2 rows = -2 A^T E + sq[n], pipelined over two n-halves
            # (separate PSUM tiles per half so the halves don't serialize
            # on tile-granular WAR hazards). The dch -> ndneg chain is
            # emitted before the x-selection path: program order is the
            # scheduler's priority, and ndneg gates the main loop.
            ps_d2a = psd.tile([AH, H], f32, tag="d2a")
            ps_d2b = psd.tile([AH, H], f32, tag="d2b")
            ps_sq0 = pss.tile([1, H], f32, tag="sq0")
            ps_sq1 = pss.tile([1, H], f32, tag="sq1")
            ps_d2h = [ps_d2a, ps_d2b]
            ps_sqh = [ps_sq0, ps_sq1]
            sq_b = sb.tile([1, B], bf16)
            d2c = sb.tile([AH, B], f32)
            dch = sb.tile([AH, B], f32)
            ndneg_b = sb.tile([AH, B], bf16)
            r1_insts = []
            with tc.high_priority():
                for h in range(2):
                    s = slice(h * H, (h + 1) * H)
                    pd = ps_d2h[h]
                    nc.tensor.matmul(
                        pd[:], emTAm2, emT_b[:, s], start=True, stop=False
                    )
                    nc.tensor.matmul(
                        ps_sqh[h][:], ones_cb[:], e2b[:, s], start=True, stop=True
                    )
                    # split the PSUM->SBUF sq copies across DVE and ACT
                    if h == 0:
                        nc.vector.tensor_copy(sq_b[:, s], ps_sqh[h][:])
                    else:
                        nc.scalar.copy(sq_b[:, s], ps_sqh[h][:])
                    r1_insts.append(
                        nc.tensor.matmul(
                            pd[:], ones_rb[:], sq_b[:, s], start=False, stop=True
                        )
                    )
                    # d2 clamp + sq[a] on ACT (Relu with per-partition bias)
                    # to keep DVE free for the ndneg / x-selection work
                    nc.scalar.activation(
                        d2c[:, s], pd[:], AF.Relu, bias=sqa_sb[:], scale=1.0
                    )
                    nc.scalar.activation(dch[:, s], d2c[:, s], AF.Sqrt)
                    nc.vector.scalar_tensor_tensor(
                        out=ndneg_b[:, s], in0=msk_t[:, s], scalar=-BIG,
                        in1=dch[:, s], op0=OP.mult, op1=OP.subtract,
                    )

            # ---- x2[a,j]: d2mT chunks -> selection matmul -> fixup
            d2mT0 = sb.tile([AH, AH], bf16)
            nc.vector.scalar_tensor_tensor(
                out=d2mT0[:], in0=ps_dt0[:], scalar=sqn0[:], in1=mskT0_t,
                op0=OP.add, op1=OP.mult,
            )
            d2mT1 = sb.tile([AH, AH], bf16)
            nc.vector.scalar_tensor_tensor(
                out=d2mT1[:], in0=ps_dt1[:], scalar=sqn1[:], in1=mskT1_t,
                op0=OP.add, op1=OP.mult,
            )
            ps_xsel = pss.tile([AH, KP], f32, tag="xsel")
            xm0 = nc.tensor.matmul(ps_xsel[:], d2mT0[:], E0_t, start=True, stop=False)
            nc.tensor.matmul(ps_xsel[:], d2mT1[:], E1_t, start=False, stop=True)
            # keep the PE queue on the critical d2 chain: the selection
            # matmuls have ~2us of slack, the rank-1s gate the main loop
            bass._add_dep_helper(
                xm0.ins, r1_insts[-1].ins, sync=True, reason="pe order"
            )
            x2a = sb.tile([AH, KP], f32)
            nc.vector.scalar_tensor_tensor(
                out=x2a[:], in0=exist_t, scalar=sqa_sb[:], in1=ps_xsel[:],
                op0=OP.mult, op1=OP.add,
            )
            x2b = sb.tile([AH, KP], f32)
            nc.vector.tensor_scalar_max(x2b[:], x2a[:], 0.0)
            xsq = sb.tile([AH, KP], f32)
            nc.scalar.activation(xsq[:], x2b[:], AF.Sqrt)
            xall = sb.tile([AH, KP], f32)
            nc.vector.scalar_tensor_tensor(
                out=xall[:], in0=xsq[:], scalar=MARGIN, in1=vm_t,
                op0=OP.add, op1=OP.mult,
            )
            xneg = sb.tile([AH, KP], f32)
            nc.vector.tensor_scalar_mul(xneg[:], xall[:], -1.0)

            # ---- main loop: ACT relu, PE row-accumulating matmul for the
            # relu sum, DVE+GpSimd is_gt fused-accum for the count
            cacc = sb.tile([AH, KP], f32)
            ps_r = pss.tile([1, B], f32, tag="pr")
            for j in range(KP):
                t = jt.tile([AH, B], bf16, tag="t")
                nc.scalar.activation(
                    t[:], ndneg_b[:], AF.Relu, bias=xall[:, j : j + 1], scale=1.0
                )
                nc.tensor.matmul(
                    ps_r[:], ones_cb[:], t[:], start=(j == 0), stop=(j == KP - 1)
                )
                g = junk.tile([AH, B], bf16, tag="g")
                nc.vector.tensor_scalar(
                    out=g[:], in0=ndneg_b[:], scalar1=xneg[:, j : j + 1],
                    scalar2=None, op0=OP.is_gt, op1=OP.add,
                    accum_out=cacc[:, j : j + 1],
                )

            # ---- final: all-DVE count fold via 32x32 block transpose
            # (no PE work after the loop -- the static scheduler would
            # hoist it between loop matmuls and stall the PE queue).
            # ccol -> cpad col 0; block transpose spreads each 32-row
            # group onto one partition row {0,32,64,96}; free-reduce
            # leaves 4 partials; a partition-strided DMA emits them.
            nc.vector.reduce_sum(cpad[:, 0:1], cacc[:], axis=X)
            ctr = sb.tile([AH, 32], f32)
            nc.vector.transpose(ctr[:], cpad[:])
            nc.vector.reduce_sum(c4[:, 0:1], ctr[:], axis=X)
            nc.vector.reduce_sum(c4[0:1, 1:2], ps_r[:], axis=X)
            nc.sync.dma_start(out=out[:], in_=c4[0:128:32, 0:2])

    return nc


def _legalize_waits(bir: bytes) -> bytes:
    """walrus codegen in this toolchain allows only one sync-wait per
    instruction; split extra waits into standalone EventSemaphore insts."""
    import json

    m = json.loads(bir)
    for fn in m["functions"]:
        for bb in fn["blocks"]:
            new = []
            for inst in bb["instructions"]:
                si = inst.get("sync_info")
                if si and si.get("on_wait") and len(si["on_wait"]) > 1:
                    waits = si["on_wait"]
                    for j, w in enumerate(waits[:-1]):
                        new.append(
                            {
                                "engine": inst["engine"],
                                "ins": [],
                                "outs": [],
                                "name": f"{inst['name']}-w{j}",
                                "opcode": "EventSemaphore",
                                "sync_info": {"on_update": [], "on_wait": [w]},
                            }
                        )
                    si["on_wait"] = [waits[-1]]
                new.append(inst)
            bb["instructions"] = new
    return json.dumps(m).encode()


def _get_nc(KP):
    key = ("nc", KP)
    if key not in _CACHE:
        nc = _build_bass(KP)
        orig = nc.to_json_bytes
        nc.to_json_bytes = lambda: _legalize_waits(orig())
        _CACHE[key] = nc
    return _CACHE[key]


def _prep(idtys):
    """Host-side index work: stable sort by id, group geometry."""
    ids = np.asarray(idtys).astype(np.int64).reshape(B)
    order = np.argsort(ids, kind="stable")
    ids_sorted = ids[order]
    g_start = np.zeros(B, np.int64)
    g_size = np.zeros(B, np.int64)
    _, starts, counts = np.unique(ids_sorted, return_index=True, return_counts=True)
    for s, c in zip(starts, counts):
        g_start[s : s + c] = s
        g_size[s : s + c] = c
    rank_sorted = np.arange(B) - g_start
    smax = int(counts.max())
    return order, ids_sorted, rank_sorted, g_size, smax


def make_in_maps(embs: np.ndarray, idtys: np.ndarray):
    import ml_dtypes

    bf = ml_dtypes.bfloat16
    embs = np.ascontiguousarray(np.asarray(embs, dtype=np.float32))
    order, ids_sorted, rank_sorted, g_size, smax = _prep(idtys)
    KP = max((smax + 1) // 2, 1)
    idx = np.arange(B)
    in_maps = []
    for c in range(NCORES):
        b, par = c // 2, c % 2
        spos = (idx - AOFF + 128 * b) % B   # sorted position at rot position i
        rot = order[spos]                   # original sample at rot position i
        ids_rot = ids_sorted[spos]
        rank_rot = rank_sorted[spos]
        size_rot = g_size[spos]
        emT = np.ascontiguousarray(embs[rot].T.astype(bf))   # [D, B]
        a_sl = slice(AOFF, AOFF + AH)
        emN = embs[rot[0:256]]                                # [256, D]
        emA = embs[rot[a_sl]]                                 # [AH, D]
        emX = np.concatenate([emN[0:128], emN[128:256], emA], axis=1)
        mask = (ids_rot[a_sl][:, None] == ids_rot[None, :]).astype(np.float32)
        maskT = (ids_rot[0:256][:, None] == ids_rot[a_sl][None, :]).astype(bf)
        E = np.zeros((256, KP), np.float32)
        r256 = rank_rot[:256]
        sel = (r256 % 2 == par) & (r256 // 2 < KP)
        E[np.nonzero(sel)[0], r256[sel] // 2] = 1.0
        ra, sa = rank_rot[a_sl], size_rot[a_sl]
        rk = 2 * np.arange(KP)[None, :] + par               # [1, KP]
        vm = ((rk < sa[:, None]) & (rk != ra[:, None])).astype(np.float32)
        exist = (rk < sa[:, None]).astype(np.float32)
        ebm = np.concatenate(
            [E[:128].astype(bf), E[128:256].astype(bf), maskT[0:128], maskT[128:256]],
            axis=1,
        )
        auxm = np.concatenate([vm, exist], axis=1).astype(np.float32)
        in_maps.append(
            {
                "emT": emT,
                "msk": np.ascontiguousarray(mask),
                "emX": np.ascontiguousarray(emX.astype(np.float32)),
                "ebm": np.ascontiguousarray(ebm),
                "aux": np.ascontiguousarray(auxm),
            }
        )
    return in_maps, KP


def combine(results):
    total = 0.0
    count = 0.0
    for r in results:
        o = np.asarray(r["out"], dtype=np.float64)
        total += o[0, 1]
        count += o[:, 0].sum()
    loss = np.float32(total / (count + 1e-16))
    return np.array(loss, dtype=np.float32)


def kernel(embs: np.ndarray, idtys: np.ndarray) -> np.ndarray:
    from concourse import bass_utils

    in_maps, KP = make_in_maps(np.asarray(embs), np.asarray(idtys))
    nc = _get_nc(KP)
    res = bass_utils.run_bass_kernel_spmd(nc, in_maps, list(range(NCORES)))
    return combine(res.results)
